# revision 50
# baseline (speedup 1.0000x reference)
"""Trainium2 Bass kernel for DifferentiableLandmarkDetector (top-k soft-argmax).

Full input: heatmap [2, 16, 96, 128, 128] f32.  For each of the 32 (B, C)
slices: top-64 over the flattened 1,572,864-voxel volume, temperature softmax
over the 64 values, probability-weighted (d, h, w) coordinate sum -> [2,16,3].

Strategy (memory-bound regime):
  - Shard the 32 independent (B,C) slices across 8 cores (4 slices = 25.2MB
    per core, contiguous in HBM).
  - Device kernel: stream the shard through SBUF in 1MB tiles (tapered
    768KB/512KB tail tiles so the final DVE reduces are short) on the SP
    HWDGE ring and
    max-reduce every 64 contiguous voxels (DVE tensor_reduce) into SBUF;
    the 393KB of group maxes go out as a bulk DMA that overlaps the stream
    tail plus a tiny 16KB write after the last reduce.  This is the single
    full read of HBM (the roofline pass); measured ~76us/core vs the ~70us
    contended-HBM floor (gapless 416GB/s DMA stream + ~7us fixed preamble
    + ~6us tail).
  - Host epilogue (O(100KB) of data): at most 64 groups can contain a top-64
    element (each such group's max >= the 64th largest value), so the top-128
    groups by group-max provably contain the entire top-64 set.  Gather those
    128*64 candidates from the input, exact top-64 (jax.lax.top_k tie
    semantics), softmax + coordinate decode in numpy.

Perf notes (measured on HW, min over repeated runs; runs vary ~78-92us with
HBM-stack contention from sibling cores):
  - gm out-DMAs inside the loop stall input loads (shared HWDGE sem lanes /
    SP FIFO ordering) -> gm writes go on the scalar-engine ring, and the
    bulk/tail split uses two separate SBUF tiles so the bulk write has no
    WAR hazard against the tail reduces (a single shared tile serializes
    and costs ~11us).
  - Uniform 1MB tiles beat 2MB/4MB and tapered mixes; sub-1MB degrades the
    descriptor stream.
  - Walrus allows only 1 sync-wait per DMA/compute instruction; building via
    bacc.Bacc (generate_event_semaphores splits waits) is required.
"""

import sys

import numpy as np

if "/opt/trn_rl_repo" not in sys.path:
    sys.path.insert(0, "/opt/trn_rl_repo")

TEMPERATURE = 0.1
TOPK = 64
B, C, D, H, W = 2, 16, 96, 128, 128
VOX = D * H * W                          # 1,572,864 voxels per (B,C) slice
N_CORES = 8
SLICES_PER_CORE = (B * C) // N_CORES     # 4
CORE_ELEMS = SLICES_PER_CORE * VOX       # 6,291,456
P = 128                                  # SBUF partitions
GROUP = 64                               # contiguous voxels per group-max
GROUPS_PER_SLICE = VOX // GROUP          # 24,576
N_GROUPS = CORE_ELEMS // GROUP           # 98,304 per core
TOP_GROUPS = 128                         # >= 64 guarantees exactness

# Free-dim widths of the streamed tiles.  Uniform 1MB tiles measured fastest
# (vs 2MB/4MB tiles and head/tail-tapered mixes): the HWDGE descriptor stream
# pipelines back-to-back at line rate and fills/drains quickly.
TILE_WIDTHS = [2048] * 22 + [1536] * 2 + [1024]
assert sum(TILE_WIDTHS) * P == CORE_ELEMS

# Set by a caller (e.g. test harness) to profile; LAST_RESULTS then holds the
# BassKernelResults with exec_time_ns.
PROFILE = False
LAST_RESULTS = None

_nc_cache = None


def _build_nc():
    global _nc_cache
    if _nc_cache is not None:
        return _nc_cache
    from concourse import bacc, mybir
    from concourse.tile import TileContext

    nc = bacc.Bacc()
    x = nc.declare_dram_parameter(
        "x", [CORE_ELEMS], mybir.dt.float32, isOutput=False
    )
    gm_cols = N_GROUPS // P  # 768
    gm = nc.declare_dram_parameter(
        "gm", [P, gm_cols], mybir.dt.float32, isOutput=True
    )

    with TileContext(nc) as tc:
        with (
            tc.tile_pool(name="data", bufs=10) as pool,
            tc.tile_pool(name="gmp", bufs=1) as gpool,
        ):
            # Group maxes accumulate into two SBUF tiles: the bulk (all but
            # the last two tail tiles) and a small tail tile.  The bulk DMA
            # fires as soon as its last reduce lands -- overlapping the tail
            # of the input stream -- and only the tiny tail write plus drain
            # sits after the last reduce.  Separate tiles keep the bulk
            # DMA's read free of WAR hazards against the tail reduces.
            n_tail = 3
            n_bulk = len(TILE_WIDTHS) - n_tail
            bulk_cols = sum(w // GROUP for w in TILE_WIDTHS[:n_bulk])
            gm_bulk = gpool.tile([P, bulk_cols], mybir.dt.float32)
            gm_tail = gpool.tile([P, gm_cols - bulk_cols], mybir.dt.float32)
            eoff = 0  # element offset into x
            gcol = 0  # column offset into gm
            for ti, w in enumerate(TILE_WIDTHS):
                gw = w // GROUP
                tl = pool.tile([P, w], mybir.dt.float32, tag="data")
                src = x[eoff:eoff + P * w].rearrange("(p f) -> p f", p=P)
                nc.sync.dma_start(out=tl[:], in_=src)
                if ti < n_bulk:
                    dst = gm_bulk[:, gcol:gcol + gw]
                else:
                    dst = gm_tail[:, gcol - bulk_cols:gcol - bulk_cols + gw]
                nc.vector.tensor_reduce(
                    out=dst,
                    in_=tl[:].rearrange("p (g e) -> p g e", e=GROUP),
                    axis=mybir.AxisListType.X,
                    op=mybir.AluOpType.max,
                )
                eoff += P * w
                gcol += gw
                if ti == n_bulk - 1:
                    nc.scalar.dma_start(
                        out=gm[:, :bulk_cols], in_=gm_bulk[:]
                    )
            nc.scalar.dma_start(out=gm[:, bulk_cols:], in_=gm_tail[:])
    nc.finalize()
    _nc_cache = nc
    return nc


def kernel(heatmap) -> np.ndarray:
    global LAST_RESULTS
    from concourse.bass_utils import run_bass_kernel_spmd

    x = np.ascontiguousarray(np.asarray(heatmap), dtype=np.float32)
    assert x.shape == (B, C, D, H, W)
    x2 = x.reshape(B * C, VOX)

    nc = _build_nc()
    in_maps = [
        {"x": np.ascontiguousarray(
            x2[i * SLICES_PER_CORE:(i + 1) * SLICES_PER_CORE].reshape(-1))}
        for i in range(N_CORES)
    ]
    try:
        res = run_bass_kernel_spmd(
            nc, in_maps, list(range(N_CORES)), trace=PROFILE
        )
    except Exception:
        # one retry for transient device/runtime hiccups
        res = run_bass_kernel_spmd(
            nc, in_maps, list(range(N_CORES)), trace=PROFILE
        )
    LAST_RESULTS = res

    ecols = np.arange(GROUP)
    out = np.zeros((B * C, 3), dtype=np.float32)
    for core in range(N_CORES):
        # gm[p, cbase+q] holds the max of core-flat elems
        # [e0 + p*w + 64q, +64), i.e. core-flat group e0/64 + p*(w/64) + q,
        # for the segment starting at element offset e0 / column cbase.
        G2 = res.results[core]["gm"]  # [128, 768]
        Gf = np.empty(N_GROUPS, dtype=np.float32)
        goff = cbase = 0
        for w in TILE_WIDTHS:
            gw = w // GROUP
            Gf[goff:goff + P * gw] = G2[:, cbase:cbase + gw].reshape(-1)
            goff += P * gw
            cbase += gw
        for s in range(SLICES_PER_CORE):
            bc = core * SLICES_PER_CORE + s
            gs = Gf[s * GROUPS_PER_SLICE:(s + 1) * GROUPS_PER_SLICE]
            top_g = np.argpartition(gs, -TOP_GROUPS)[-TOP_GROUPS:]
            fpos = (top_g[:, None] * GROUP + ecols[None, :]).reshape(-1)
            vals = x2[bc, fpos]
            # descending by value, ties -> lower index (jax.lax.top_k order)
            order = np.lexsort((fpos, -vals))[:TOPK]
            v64 = vals[order].astype(np.float64)
            p64 = fpos[order]
            w = v64 / TEMPERATURE
            w -= w.max()
            ew = np.exp(w)
            probs = ew / (ew.sum() + 1e-20)
            d = p64 // (H * W)
            h = (p64 % (H * W)) // W
            wv = p64 % W
            out[bc, 0] = (probs * d).sum()
            out[bc, 1] = (probs * h).sum()
            out[bc, 2] = (probs * wv).sum()
    return out.reshape(B, C, 3)
